# revision 9
# baseline (speedup 1.0000x reference)
"""Chamfer distance kernel for Trainium2 (8 NeuronCores, data-parallel over batch).

Input : x, y float32 [16, 4096, 3]
Output: scalar float32 = mean_b [ mean_n min_m ||x_bn - y_bm||^2
                                + mean_m min_n ||x_bn - y_bm||^2 ]

Strategy (per core = 2 batches):
  Host: cast points to bf16 (x~, y~), build K=7 augmented vectors so that
        ax . ay = 2 x~.y~ - |x~|^2 - |y~|^2 = -d(x~,y~)   exactly in fp32
        (|.|^2 terms split hi/lo into two bf16 entries to avoid rounding).
  PE   : per 128-x block, 8 matmuls [7,128]^T @ [7,512] -> PSUM fp32 = -d.
  ACT  : cast PSUM fp32 -> SBUF bf16 (S tiles).
  DVE  : dirA: tensor_tensor_reduce(max of the two halves, fused row-max)
               -> rowacc[:, block]    (= -min_m d per x point)
         dirB: colrun = max(colrun, S) running over blocks.
  GPSIMD: batch finalize: partition_all_reduce over rowacc sums (add) and
          colrun (max); DVE sums -> scalar accumulator.
"""
import sys

sys.path.insert(0, "/opt/trn_rl_repo")

import numpy as np
import ml_dtypes

import concourse.bacc as bacc
import concourse.bass as bass
import concourse.bass_isa as bass_isa
import concourse.tile as tile
from concourse import mybir
from concourse.alu_op_type import AluOpType
from concourse.bass_utils import run_bass_kernel_spmd

F32 = mybir.dt.float32
BF16 = mybir.dt.bfloat16
X = mybir.AxisListType.X
A = AluOpType

B, N, D3 = 16, 4096, 3
NCORES = 8
BPC = B // NCORES           # batches per core
RB = N // 128               # 32 row blocks of x per batch
KAUG = 7                    # augmented contraction dim
HALF = N // 2               # 2048
NEG_INF = -1.0e30

import os
USE_TTR = os.environ.get("K_TTR", "0") == "1"        # fused fold+rowmax (ucode op
                                                     # rejected by HW path - keep 0)
PAR_BF16 = os.environ.get("K_PARBF16", "1") == "1"   # bf16 partition_all_reduce


def _build_nc(repeat: int = 1):
    nc = bacc.Bacc("TRN2", target_bir_lowering=False, debug=False, num_devices=NCORES)
    # ax[k, b*N + n]: [2x0, 2x1, 2x2, -x2hi, -x2lo, 1, 1]
    # ay[k, b*N + m]: [y0, y1, y2, 1, 1, -y2hi, -y2lo]
    ax_d = nc.dram_tensor("ax", [KAUG, BPC * N], BF16, kind="ExternalInput").ap()
    ay_d = nc.dram_tensor("ay", [KAUG, BPC * N], BF16, kind="ExternalInput").ap()
    out_d = nc.dram_tensor("out", [1, 1], F32, kind="ExternalOutput").ap()

    with tile.TileContext(nc) as tc:
        import contextlib
        with contextlib.ExitStack() as ctx:
            const = ctx.enter_context(tc.tile_pool(name="const", bufs=1))
            acc = ctx.enter_context(tc.tile_pool(name="acc", bufs=1))
            sp = ctx.enter_context(tc.tile_pool(name="sp", bufs=4))
            ps = ctx.enter_context(tc.tile_pool(name="ps", bufs=2, space="PSUM"))

            ax_t = const.tile([KAUG, BPC * N], BF16, name="ax_t")
            ay_t = const.tile([KAUG, BPC * N], BF16, name="ay_t")
            nc.gpsimd.dma_start(ax_t[:], ax_d[:])
            nc.gpsimd.dma_start(ay_t[:], ay_d[:])

            colrun = acc.tile([128, N], BF16, name="colrun")
            rowacc = acc.tile([128, RB], F32, name="rowacc")
            f1 = acc.tile([128, HALF], BF16, name="f1")
            f2 = acc.tile([128, HALF // 2], BF16, name="f2")
            f3 = acc.tile([128, HALF // 4], BF16, name="f3")
            cred = acc.tile([128, N], BF16, name="cred")
            stot = acc.tile([1, 1], F32, name="stot")
            s_out = acc.tile([1, 1], F32, name="s_out")

            for it in range(BPC * repeat):
                b = it % BPC
                for r in range(RB):
                    s_halves = []
                    for h in range(2):
                        psum_t = ps.tile([128, HALF], F32, name="psum", tag="psum")
                        for c in range(4):
                            mcol = b * N + h * HALF + c * 512
                            nc.tensor.matmul(
                                psum_t[:, c * 512:(c + 1) * 512],
                                ax_t[:, b * N + r * 128: b * N + (r + 1) * 128],
                                ay_t[:, mcol: mcol + 512],
                                start=True, stop=True,
                            )
                        s_h = sp.tile([128, HALF], BF16, name="s_h", tag="S")
                        nc.scalar.copy(s_h[:], psum_t[:])
                        # dirB: running per-column max (over x points)
                        cslice = colrun[:, h * HALF:(h + 1) * HALF]
                        if r == 0:
                            nc.vector.tensor_copy(cslice, s_h[:])
                        else:
                            nc.vector.tensor_tensor(cslice, cslice, s_h[:], op=A.max)
                        s_halves.append(s_h)
                    # dirA: fused fold-of-halves + row max -> rowacc[:, r]
                    if USE_TTR:
                        nc.vector.tensor_tensor_reduce(
                            out=f1[:],
                            in0=s_halves[0][:],
                            in1=s_halves[1][:],
                            scale=1.0,
                            scalar=NEG_INF,
                            op0=A.max,
                            op1=A.max,
                            accum_out=rowacc[:, r: r + 1],
                        )
                    else:
                        nc.vector.tensor_tensor(f1[:], s_halves[0][:],
                                                s_halves[1][:], op=A.max)
                        nc.vector.tensor_tensor(f2[:], f1[:, 0:HALF // 2],
                                                f1[:, HALF // 2:], op=A.max)
                        nc.vector.tensor_tensor(f3[:], f2[:, 0:HALF // 4],
                                                f2[:, HALF // 4:], op=A.max)
                        nc.vector.tensor_reduce(rowacc[:, r: r + 1], f3[:],
                                                axis=X, op=A.max)

                # ---- batch finalize ----
                rs = acc.tile([128, 1], F32, name=f"rs_{it}")
                nc.vector.reduce_sum(rs[:], rowacc[:], axis=X)
                rsr = acc.tile([128, 1], F32, name=f"rsr_{it}")
                nc.gpsimd.partition_all_reduce(rsr[:], rs[:], channels=128,
                                               reduce_op=bass_isa.ReduceOp.add)
                if PAR_BF16:
                    nc.gpsimd.partition_all_reduce(cred[:], colrun[:], channels=128,
                                                   reduce_op=bass_isa.ReduceOp.max)
                    credv = cred
                else:
                    colf = acc.tile([128, N], F32, name=f"colf_{it}", tag="colf")
                    nc.vector.tensor_copy(colf[:], colrun[:])
                    credf = acc.tile([128, N], F32, name=f"credf_{it}", tag="credf")
                    nc.gpsimd.partition_all_reduce(credf[:], colf[:], channels=128,
                                                   reduce_op=bass_isa.ReduceOp.max)
                    credv = credf
                cs = acc.tile([1, 1], F32, name=f"cs_{it}")
                nc.vector.reduce_sum(cs[:], credv[0:1, :], axis=X)
                bt = acc.tile([1, 1], F32, name=f"bt_{it}")
                nc.vector.tensor_add(bt[:], rsr[0:1, 0:1], cs[:])
                if it == 0:
                    nc.vector.tensor_copy(stot[:], bt[:])
                else:
                    nc.vector.tensor_add(stot[:], stot[:], bt[:])

            nc.scalar.mul(s_out[:], stot[:], -1.0 / (N * repeat))
            nc.gpsimd.dma_start(out_d[:], s_out[:])
    nc.compile()
    return nc


def _build_operands(x, y):
    """x,y [B,N,3] f32 -> per-core input maps (augmented bf16 layouts)."""
    x = np.ascontiguousarray(x, np.float32)
    y = np.ascontiguousarray(y, np.float32)
    bf = ml_dtypes.bfloat16
    ones = np.ones(N, np.float32)
    in_maps = []
    for core in range(NCORES):
        ax_parts, ay_parts = [], []
        for b in range(core * BPC, (core + 1) * BPC):
            xb = x[b].astype(bf).astype(np.float32)      # [N,3] perturbed pts
            yb = y[b].astype(bf).astype(np.float32)
            x2 = (xb * xb).sum(1)
            y2 = (yb * yb).sum(1)
            xh = x2.astype(bf).astype(np.float32)
            xl = x2 - xh
            yh = y2.astype(bf).astype(np.float32)
            yl = y2 - yh
            ax_parts.append(np.stack(
                [2 * xb[:, 0], 2 * xb[:, 1], 2 * xb[:, 2], -xh, -xl, ones, ones], 0))
            ay_parts.append(np.stack(
                [yb[:, 0], yb[:, 1], yb[:, 2], ones, ones, -yh, -yl], 0))
        ax = np.concatenate(ax_parts, axis=1)            # [7, BPC*N]
        ay = np.concatenate(ay_parts, axis=1)
        in_maps.append({"ax": np.ascontiguousarray(ax.astype(bf)),
                        "ay": np.ascontiguousarray(ay.astype(bf))})
    return in_maps


_NC_CACHE = {}


def _get_nc(repeat: int = 1):
    if repeat not in _NC_CACHE:
        _NC_CACHE[repeat] = _build_nc(repeat)
    return _NC_CACHE[repeat]


def kernel(x, y):
    x = np.asarray(x, dtype=np.float32)
    y = np.asarray(y, dtype=np.float32)
    assert x.shape == (B, N, D3) and y.shape == (B, N, D3)
    in_maps = _build_operands(x, y)
    nc = _get_nc(1)
    res = run_bass_kernel_spmd(nc, in_maps, core_ids=list(range(NCORES)))
    total = sum(float(res.results[i]["out"][0, 0]) for i in range(NCORES))
    return np.float32(total / B)
